# revision 1
# baseline (speedup 1.0000x reference)
"""TRN2 Bass/Tile kernel: deformable-kernel spatial attention (dense_cnn).

Per-core (pure data parallel, batch 8 over 8 cores):
  h1 = relu(conv1(x))     3x3 stride-2 64->64 as 9 tap-matmuls (K=64, fp32r)
  5x dkc:                 global-pool -> fc offsets -> gather-free bilinear
                          resample (hat functions) of the 4x4 scope kernel ->
                          depthwise 3x3 = PE diag-matmuls + DVE per-partition
                          FMAs accumulating straight into PSUM
  conv2+pixel_shuffle+conv3 fused into a 64->4 channel 3x3 conv
  att = sigmoid(logits); out = x * att (att replicated across channels with
  selector matmuls into PSUM; DVE multiplies from PSUM)
"""

import numpy as np

import concourse.bass as bass
import concourse.mybir as mybir
import concourse.tile as tile
from concourse import bacc
from concourse.bass_utils import run_bass_kernel_spmd
from concourse.masks import make_identity
from contextlib import ExitStack

f32 = mybir.dt.float32
f32r = mybir.dt.float32r
bf16 = mybir.dt.bfloat16
i32 = mybir.dt.int32
AF = mybir.ActivationFunctionType
ALU = mybir.AluOpType
AX = mybir.AxisListType

C = 64
H = 256
HH = 128
RB = 64          # interior rows per half
SLOTS = RB + 2   # + top/bottom halo row
WCOL = HH + 2    # zero gutter columns at 0 and 129
NL = 5

TAPS = [(t // 3 - 1, t % 3 - 1) for t in range(9)]  # t = 3*ty+tx -> (dy, dx)

# dkc tap split: these taps run as PE diag-matmuls, the rest as DVE FMAs.
N_PE_DKC = 4
PE_TAPS = [0, 2, 6, 8, 3, 5, 1, 7, 4][:N_PE_DKC]
DVE_TAPS = [t for t in range(9) if t not in PE_TAPS]


def _ap(a, extra_off, dims):
    return bass.AP(tensor=a.tensor, offset=a.offset + extra_off, ap=dims)


def build_nc():
    nc = bacc.Bacc("TRN2", target_bir_lowering=False, debug=False)
    x_d = nc.dram_tensor("x", [C, H, H], f32, kind="ExternalInput").ap()
    w1_d = nc.dram_tensor("conv1_w", [C, C, 3, 3], f32, kind="ExternalInput").ap()
    b1_d = nc.dram_tensor("conv1_b", [C], f32, kind="ExternalInput").ap()
    dkw_d = nc.dram_tensor("dkc_w", [NL, C, 1, 4, 4], f32, kind="ExternalInput").ap()
    dkb_d = nc.dram_tensor("dkc_b", [NL, C], f32, kind="ExternalInput").ap()
    fcw_d = nc.dram_tensor("dkc_fc_w", [NL, 18, C], f32, kind="ExternalInput").ap()
    fcb_d = nc.dram_tensor("dkc_fc_b", [NL, 18], f32, kind="ExternalInput").ap()
    w2_d = nc.dram_tensor("conv2_w", [4 * C, C, 3, 3], f32, kind="ExternalInput").ap()
    b2_d = nc.dram_tensor("conv2_b", [4 * C], f32, kind="ExternalInput").ap()
    w3_d = nc.dram_tensor("conv3_w", [1, C, 1, 1], f32, kind="ExternalInput").ap()
    b3_d = nc.dram_tensor("conv3_b", [1], f32, kind="ExternalInput").ap()
    o_d = nc.dram_tensor("out", [C, H, H], f32, kind="ExternalOutput").ap()

    with tile.TileContext(nc) as tc:
        with ExitStack() as ctx:
            _kernel(ctx, tc, nc, x_d, w1_d, b1_d, dkw_d, dkb_d, fcw_d, fcb_d,
                    w2_d, b2_d, w3_d, b3_d, o_d)
    nc.compile()
    return nc


def _kernel(ctx, tc, nc, x_d, w1_d, b1_d, dkw_d, dkb_d, fcw_d, fcb_d,
            w2_d, b2_d, w3_d, b3_d, o_d):
    persist = ctx.enter_context(tc.tile_pool(name="persist", bufs=1))
    hpool = ctx.enter_context(tc.tile_pool(name="h", bufs=2))
    bandp = ctx.enter_context(tc.tile_pool(name="band", bufs=2))
    small = ctx.enter_context(tc.tile_pool(name="small", bufs=4))
    diagp = ctx.enter_context(tc.tile_pool(name="diag", bufs=2))
    outp = ctx.enter_context(tc.tile_pool(name="outb", bufs=2))
    psum = ctx.enter_context(tc.tile_pool(name="psum", bufs=2, space="PSUM"))
    psmall = ctx.enter_context(tc.tile_pool(name="psmall", bufs=2, space="PSUM"))
    pattn = ctx.enter_context(tc.tile_pool(name="pattn", bufs=2, space="PSUM"))

    # ---------------- one-time setup ----------------
    w1stage = persist.tile([C, 9, C], f32)
    for t in range(9):
        nc.sync.dma_start(out=w1stage[:, t, :],
                          in_=_ap(w1_d, t, [[9, C], [576, C]]))
    w1t = persist.tile([128, 9, C], bf16)
    nc.scalar.activation(w1t[0:C, :, :], w1stage[:], AF.Copy, bias=0.0, scale=1.0)
    nc.gpsimd.dma_start(out=w1t[C:128, :, :], in_=w1t[0:C, :, :])

    i128 = persist.tile([128, C], f32)
    make_identity(nc, i128[0:C, :])
    nc.gpsimd.dma_start(out=i128[C:128, :], in_=i128[0:C, :])

    zrow = persist.tile([128, H], f32)
    nc.vector.memset(zrow[:], 0.0)

    biases = persist.tile([128, 6], f32)
    nc.sync.dma_start(out=biases[0:C, 0:1], in_=b1_d.unsqueeze(-1))
    for i in range(NL):
        nc.sync.dma_start(out=biases[0:C, 1 + i:2 + i], in_=dkb_d[i].unsqueeze(-1))
    nc.gpsimd.dma_start(out=biases[C:128, :], in_=biases[0:C, :])

    w2dt = persist.tile([16, NL, C], f32r)
    fcwt = persist.tile([C, NL, 18], f32r)
    fcb = persist.tile([1, NL, 18], f32)
    for i in range(NL):
        nc.sync.dma_start(out=w2dt[:, i, :],
                          in_=_ap(dkw_d.bitcast(f32r), i * 1024, [[1, 16], [16, C]]))
        nc.sync.dma_start(out=fcwt[:, i, :],
                          in_=_ap(fcw_d.bitcast(f32r), i * 18 * C, [[1, C], [C, 18]]))
        nc.sync.dma_start(out=fcb[:, i, :], in_=fcb_d[i:i + 1, :])

    it = small.tile([16, 9], i32, tag="it")
    by16 = persist.tile([16, 9], f32)
    bx16 = persist.tile([16, 9], f32)
    nc.gpsimd.iota(it[:], pattern=[[1, 3], [0, 3]], base=0, channel_multiplier=0)
    nc.vector.tensor_copy(by16[:], it[:])
    nc.vector.tensor_scalar_add(by16[:], by16[:], 0.5)
    nc.gpsimd.iota(it[:], pattern=[[0, 3], [1, 3]], base=0, channel_multiplier=0)
    nc.vector.tensor_copy(bx16[:], it[:])
    nc.vector.tensor_scalar_add(bx16[:], bx16[:], 0.5)
    itp = small.tile([16, 1], i32, tag="itp")
    idx16 = small.tile([16, 1], f32, tag="idx16")
    ky16 = persist.tile([16, 1], f32)
    kx16 = persist.tile([16, 1], f32)
    nc.gpsimd.iota(itp[:], pattern=[[0, 1]], base=0, channel_multiplier=1)
    nc.vector.tensor_copy(idx16[:], itp[:])
    st16 = small.tile([1, 16], i32, tag="st16")
    nc.gpsimd.iota(st16[:], pattern=[[1, 4], [0, 4]], base=0, channel_multiplier=0)
    stf = small.tile([1, 16], f32, tag="stf")
    nc.vector.tensor_copy(stf[:], st16[:])
    nc.gpsimd.dma_start(out=ky16[:], in_=_ap(stf, 0, [[1, 16], [16, 1]]))
    nc.vector.scalar_tensor_tensor(kx16[:], ky16[:], -4.0, idx16[:], ALU.mult, ALU.add)

    # fused conv2/conv3: W2fT[ci, t*4+j] = sum_c conv2_w[4c+j, ci, t] * conv3_w[c]
    w3sb = persist.tile([C, 4], f32r)
    nc.gpsimd.dma_start(out=w3sb[:].unsqueeze(-1), in_=_ap(w3_d.bitcast(f32r), 0, [[1, C], [0, 4], [1, 1]]))
    c2wj = persist.tile([C, 4, 576], f32r)
    for j in range(4):
        nc.sync.dma_start(out=c2wj[:, j, :],
                          in_=_ap(w2_d.bitcast(f32r), j * 576, [[4 * 576, C], [1, 576]]))
    w2f_ps = psmall.tile([C, 144], f32, tag="sp")
    for t in range(9):
        for j in range(4):
            lhsT = _ap(c2wj, j * 576 + t, [c2wj.ap[0], [9, C]])
            k = (t * 4 + j) * 4
            nc.tensor.matmul(w2f_ps[:, k:k + 4], lhsT, w3sb[:],
                             start=True, stop=True)
    w2ft = persist.tile([128, 36], bf16)
    nc.scalar.activation(w2ft[0:C, :],
                         _ap(w2f_ps, 0, [w2f_ps.ap[0], [4, 36]]),
                         AF.Copy, bias=0.0, scale=1.0)
    nc.gpsimd.dma_start(out=w2ft[C:128, :], in_=w2ft[0:C, :])
    c2bj = persist.tile([C, 4], f32r)
    nc.sync.dma_start(out=c2bj[:], in_=_ap(b2_d.bitcast(f32r), 0, [[4, C], [1, 4]]))
    b2f_ps = psmall.tile([4, 4], f32, tag="sp")
    nc.tensor.matmul(b2f_ps[:], c2bj[:], w3sb[:], start=True, stop=True)
    b3b = small.tile([4, 1], f32, tag="b3b")
    nc.gpsimd.dma_start(out=b3b[:], in_=_ap(b3_d, 0, [[0, 4], [1, 1]]))
    b2f = persist.tile([4, 1], f32)
    nc.scalar.activation(b2f[:], b2f_ps[:, 0:1], AF.Copy, bias=0.0, scale=1.0)
    nc.vector.tensor_add(b2f[:], b2f[:], b3b[:])

    seljf = persist.tile([4, 4 * C], f32)
    nc.gpsimd.memset(seljf[:], 0.0)
    for j in range(4):
        nc.gpsimd.affine_select(out=seljf[:, j * C:(j + 1) * C],
                                in_=seljf[:, j * C:(j + 1) * C],
                                pattern=[[0, C]], compare_op=ALU.not_equal,
                                fill=1.0, base=-j, channel_multiplier=1)
    selj = persist.tile([4, 4 * C], bf16)
    nc.scalar.activation(selj[:], seljf[:], AF.Copy, bias=0.0, scale=1.0)
    ones116f = persist.tile([1, 16], f32)
    nc.vector.memset(ones116f, 1.0)
    ones116 = persist.tile([1, 16], f32r)
    nc.scalar.activation(ones116[:], ones116f[:], AF.Copy, bias=0.0, scale=1.0)

    # ---------------- h tensors ----------------
    def new_h():
        h = hpool.tile([128, SLOTS, WCOL], bf16, tag="h")
        nc.scalar.activation(h[0:C, 0, :], zrow[0:C, 0:WCOL], AF.Copy, bias=0.0, scale=1.0)
        nc.scalar.activation(h[C:128, SLOTS - 1, :], zrow[C:128, 0:WCOL], AF.Copy, bias=0.0, scale=1.0)
        zc = _ap(zrow, 0, [zrow.ap[0], [1, SLOTS], [1, 1]])
        nc.scalar.activation(h[:, :, 0:1], zc, AF.Copy, bias=0.0, scale=1.0)
        nc.scalar.activation(h[:, :, WCOL - 1:WCOL], zc, AF.Copy, bias=0.0, scale=1.0)
        return h

    def halo_fix(h):
        nc.gpsimd.dma_start(out=h[C:128, 0, :], in_=h[0:C, RB, :])
        nc.gpsimd.dma_start(out=h[0:C, SLOTS - 1, :], in_=h[C:128, 1, :])

    # ---------------- conv1 ----------------
    h1 = new_h()
    pp1 = small.tile([128, 16], f32, tag="pp")
    for b in range(16):
        bandf = bandp.tile([128, 9, H], f32, tag="bandf")
        r0 = 8 * b - 1
        lo = max(r0, 0)
        lo_skip = lo - r0
        if lo_skip:
            nc.vector.tensor_copy(bandf[0:C, 0, :], zrow[0:C, :])
        nc.sync.dma_start(out=bandf[0:C, lo_skip:9, :], in_=x_d[:, lo:r0 + 9, :])
        r1 = r0 + 128
        hi = min(r1 + 9, H) - r1
        if hi < 9:
            nc.vector.tensor_copy(bandf[C:128, hi, :], zrow[C:128, :])
        nc.sync.dma_start(out=bandf[C:128, 0:hi, :], in_=x_d[:, r1:r1 + hi, :])
        band = bandp.tile([128, 9, H + 2], bf16, tag="band")
        zc9 = _ap(zrow, 0, [zrow.ap[0], [1, 9], [1, 1]])
        nc.scalar.activation(band[:, :, 0:1], zc9, AF.Copy, bias=0.0, scale=1.0)
        nc.vector.tensor_copy(band[:, :, 1:H + 1], bandf[:])
        for g in range(2):
            gl, gh = (0, C) if g == 0 else (C, 128)
            ps = psum.tile([128, 4, HH], f32, tag="cps")
            for t in range(9):
                dy, dx = TAPS[t]
                ky, kx = dy + 1, dx + 1
                rhs = band[gl:gh, ky:ky + 7:2, kx:kx + 255:2]
                nc.tensor.matmul(ps[gl:gh, :, :], w1t[gl:gh, t, :], rhs,
                                 start=(t == 0), stop=(t == 8))
            s0 = 1 + 4 * b
            nc.scalar.activation(h1[gl:gh, s0:s0 + 4, 1:HH + 1], ps[gl:gh],
                                 AF.Relu, bias=biases[gl:gh, 0:1], scale=1.0,
                                 accum_out=pp1[gl:gh, b:b + 1])
    halo_fix(h1)

    # ---------------- dkc layers ----------------
    h_cur, pp_cur = h1, pp1
    for li in range(NL):
        red = small.tile([128, 1], f32, tag="red")
        nc.vector.tensor_reduce(out=red[:], in_=pp_cur[:], axis=AX.X, op=ALU.add)
        tmp64 = small.tile([C, 1], f32, tag="t64")
        nc.gpsimd.dma_start(out=tmp64[:], in_=red[C:128, :])
        featf = small.tile([C, 1], f32, tag="featf")
        nc.vector.tensor_add(featf[:], red[0:C, :], tmp64[:])
        feat = small.tile([C, 1], f32r, tag="feat")
        nc.scalar.activation(feat[:], featf[:], AF.Copy, bias=0.0, scale=1.0 / 16384.0)
        offp = psmall.tile([1, 18], f32, tag="sp")
        nc.tensor.matmul(offp[:], feat[:], fcwt[:, li, :], start=True, stop=True)
        offf = small.tile([1, 18], f32, tag="offf")
        nc.vector.tensor_add(offf[:], offp[:], fcb[:, li, :])
        off = small.tile([1, 18], f32r, tag="off")
        nc.scalar.activation(off[:], offf[:], AF.Copy, bias=0.0, scale=1.0)
        offr_ps = psmall.tile([16, 18], f32, tag="sp")
        nc.tensor.matmul(offr_ps[:], ones116[:], off[:], start=True, stop=True)
        phiy = small.tile([16, 9], f32, tag="phiy")
        phix = small.tile([16, 9], f32, tag="phix")
        kintf = small.tile([16, 10], f32, tag="kintf")
        kint = small.tile([16, 10], f32r, tag="kint")
        nc.vector.memset(kintf[:, 9:10], 0.0)
        nc.vector.tensor_add(phiy[:], offr_ps[:, 0:9], by16[:])
        nc.vector.tensor_scalar(phiy[:], phiy[:], ky16[:], None, ALU.subtract)
        nc.scalar.activation(phiy[:], phiy[:], AF.Abs, bias=0.0, scale=1.0)
        nc.scalar.activation(phiy[:], phiy[:], AF.Relu, bias=1.0, scale=-1.0)
        nc.vector.tensor_add(phix[:], offr_ps[:, 9:18], bx16[:])
        nc.vector.tensor_scalar(phix[:], phix[:], kx16[:], None, ALU.subtract)
        nc.scalar.activation(phix[:], phix[:], AF.Abs, bias=0.0, scale=1.0)
        nc.scalar.activation(phix[:], phix[:], AF.Relu, bias=1.0, scale=-1.0)
        nc.vector.tensor_tensor(kintf[:, 0:9], phiy[:], phix[:], ALU.mult)
        nc.scalar.activation(kint[:], kintf[:], AF.Copy, bias=0.0, scale=1.0)
        samp_ps = psmall.tile([C, 10], f32, tag="sp")
        nc.tensor.matmul(samp_ps[:], w2dt[:, li, :], kint[:], start=True, stop=True)
        samp = small.tile([128, 9], f32, tag="samp")
        nc.scalar.activation(samp[0:C, :], samp_ps[:, 0:9], AF.Copy, bias=0.0, scale=1.0)
        nc.gpsimd.dma_start(out=samp[C:128, :], in_=samp[0:C, :])
        diagf = diagp.tile([128, 9, C], f32, tag="diagf")
        diag = diagp.tile([128, 9, C], bf16, tag="diag")
        for t in PE_TAPS:
            nc.vector.tensor_scalar(diagf[:, t, :], i128[:], samp[:, t:t + 1],
                                    None, ALU.mult)
            nc.scalar.activation(diag[:, t, :], diagf[:, t, :], AF.Copy,
                                 bias=0.0, scale=1.0)

        h_nxt = new_h()
        if li < NL - 1:
            pp_nxt = small.tile([128, 16], f32, tag="pp")
        else:
            pp_nxt = None
        for g in range(2):
            gl, gh = (0, C) if g == 0 else (C, 128)
            for b in range(16):
                s0 = 1 + 4 * b
                ps = psum.tile([128, 4, HH], f32, tag="cps")

                def win(src, t):
                    dy, dx = TAPS[t]
                    return (src[gl:gh, s0 + dy:s0 + dy + 4, 1 + dx:1 + dx + HH],
                            ps[gl:gh, :, :])

                for ti, t in enumerate(PE_TAPS):
                    rhs, out_ap = win(h_cur, t)
                    nc.tensor.matmul(out_ap, diag[gl:gh, t, :], rhs,
                                     start=(ti == 0), stop=(ti == N_PE_DKC - 1))
                for t in DVE_TAPS:
                    rhs, out_ap = win(h_cur, t)
                    nc.vector.scalar_tensor_tensor(out_ap, rhs,
                                                   samp[gl:gh, t:t + 1], out_ap,
                                                   ALU.mult, ALU.add)
                if pp_nxt is not None:
                    nc.scalar.activation(h_nxt[gl:gh, s0:s0 + 4, 1:HH + 1], ps[gl:gh],
                                         AF.Relu, bias=biases[gl:gh, 1 + li:2 + li],
                                         scale=1.0, accum_out=pp_nxt[gl:gh, b:b + 1])
                else:
                    nc.scalar.activation(h_nxt[gl:gh, s0:s0 + 4, 1:HH + 1], ps[gl:gh],
                                         AF.Relu, bias=biases[gl:gh, 1 + li:2 + li],
                                         scale=1.0)
        halo_fix(h_nxt)
        h_cur, pp_cur = h_nxt, pp_nxt

    # ---------------- fused conv2' -> sigmoid att ----------------
    att = persist.tile([4, HH, HH], bf16)
    for g in range(2):
        gl, gh = (0, C) if g == 0 else (C, 128)
        for b in range(16):
            s0 = 1 + 4 * b
            ps = psum.tile([128, 4, HH], f32, tag="cps")
            for t in range(9):
                dy, dx = TAPS[t]
                rhs = h_cur[gl:gh, s0 + dy:s0 + dy + 4, 1 + dx:1 + dx + HH]
                nc.tensor.matmul(ps[0:4, :, :], w2ft[gl:gh, t * 4:(t + 1) * 4], rhs,
                                 start=(t == 0), stop=(t == 8))
            yy0 = RB * g + 4 * b
            nc.scalar.activation(att[:, yy0:yy0 + 4, :], ps[0:4],
                                 AF.Sigmoid, bias=b2f[:], scale=1.0)

    # ---------------- final: out = x * att ----------------
    for b in range(32):
        y0 = 4 * b
        xb = outp.tile([128, 4, H], f32, tag="xb")
        nc.sync.dma_start(out=xb[0:C, :, :], in_=x_d[:, y0:y0 + 4, :])
        nc.sync.dma_start(out=xb[C:128, :, :], in_=x_d[:, 128 + y0:128 + y0 + 4, :])
        pa = pattn.tile([128, 4, 2, HH], f32, tag="pa")
        for gx in range(2):
            gl, gh = (0, C) if gx == 0 else (C, 128)
            for j in range(4):
                rhs = att[0:4, 64 * gx + 2 * b:64 * gx + 2 * b + 2, :]
                nc.tensor.matmul(pa[gl:gh, j, :, :], selj[:, j * C:(j + 1) * C],
                                 rhs, start=True, stop=True, skip_group_check=True)
        ob = outp.tile([128, 4, H], f32, tag="ob")
        for j in range(4):
            dy, dx = j // 2, j % 2
            xv = _ap(xb, dy * H + dx, [xb.ap[0], [2 * H, 2], [2, HH]])
            ov = _ap(ob, dy * H + dx, [ob.ap[0], [2 * H, 2], [2, HH]])
            nc.vector.tensor_tensor(ov, xv, pa[:, j, :, :], ALU.mult)
        nc.sync.dma_start(out=o_d[:, y0:y0 + 4, :], in_=ob[0:C, :, :])
        nc.sync.dma_start(out=o_d[:, 128 + y0:128 + y0 + 4, :], in_=ob[C:128, :, :])


_NC_CACHE = {}


def kernel(**inputs):
    if "nc" not in _NC_CACHE:
        _NC_CACHE["nc"] = build_nc()
    nc = _NC_CACHE["nc"]
    names = ["conv1_w", "conv1_b", "dkc_w", "dkc_b", "dkc_fc_w", "dkc_fc_b",
             "conv2_w", "conv2_b", "conv3_w", "conv3_b"]
    shared = {n: np.ascontiguousarray(np.asarray(inputs[n], dtype=np.float32))
              for n in names}
    x = np.asarray(inputs["x"], dtype=np.float32)
    in_maps = [dict(shared, x=np.ascontiguousarray(x[i])) for i in range(8)]
    r = run_bass_kernel_spmd(nc, in_maps, list(range(8)))
    _NC_CACHE["last_result"] = r
    return np.stack([r.results[i]["out"] for i in range(8)]).astype(np.float32)



# revision 7
# speedup vs baseline: 1.4078x; 1.4078x over previous
"""TRN2 Bass/Tile kernel: deformable-kernel spatial attention (dense_cnn).

Per-core (pure data parallel, batch 8 over 8 cores):
  x cached in SBUF as fp16 (single HBM read), spatially split into two
  64-partition groups (top/bottom half) so every matmul runs K=128 with
  block-diagonal weights.
  h1 = relu(conv1(x))     3x3 stride-2 64->64, 9 tap-matmuls, 8-row bands
  5x dkc:                 global-pool -> fc offsets -> hat-function resample
                          of the 4x4 scope kernel -> depthwise 3x3 as 9
                          diagonal matmuls accumulating in PSUM
  conv2+pixel_shuffle+conv3 fused into a 64->4 channel 3x3 conv
  att replicated across channels with K=8 selector matmuls; final
  out = x * att on DVE from the fp16 x cache.
"""

import numpy as np

import concourse.bass as bass
import concourse.mybir as mybir
import concourse.tile as tile
from concourse import bacc
from concourse.bass_utils import run_bass_kernel_spmd
from concourse.masks import make_identity
from contextlib import ExitStack

f32 = mybir.dt.float32
f32r = mybir.dt.float32r
f16 = mybir.dt.float16
AF = mybir.ActivationFunctionType
ALU = mybir.AluOpType
AX = mybir.AxisListType

C = 64
H = 256
HH = 128
RB = 64          # interior feature rows per half
SLOTS = RB + 2   # + top/bottom halo row
WCOL = HH + 2    # zero gutter columns at 0 and 129
XSL = 130        # x-cache slots per half (128 interior + halo at 0, unused 129)
XW = H + 2       # x-cache cols with zero gutters
NL = 5
BR = 4           # output rows per band (moving operand max 512 elements)
NB = RB // BR    # bands per phase
YY = BR // 2     # att rows per final band
CH = 16          # x rows per load chunk (per half)

TAPS = [(t // 3 - 1, t % 3 - 1) for t in range(9)]  # t = 3*ty+tx -> (dy, dx)


def _ap(a, extra_off, dims):
    return bass.AP(tensor=a.tensor, offset=a.offset + extra_off, ap=dims)


def build_nc():
    nc = bacc.Bacc("TRN2", target_bir_lowering=False, debug=False)
    x_d = nc.dram_tensor("x", [C, H, H], f32, kind="ExternalInput").ap()
    w1_d = nc.dram_tensor("conv1_w", [C, C, 3, 3], f32, kind="ExternalInput").ap()
    b1_d = nc.dram_tensor("conv1_b", [C], f32, kind="ExternalInput").ap()
    dkw_d = nc.dram_tensor("dkc_w", [NL, C, 1, 4, 4], f32, kind="ExternalInput").ap()
    dkb_d = nc.dram_tensor("dkc_b", [NL, C], f32, kind="ExternalInput").ap()
    fcw_d = nc.dram_tensor("dkc_fc_w", [NL, 18, C], f32, kind="ExternalInput").ap()
    fcb_d = nc.dram_tensor("dkc_fc_b", [NL, 18], f32, kind="ExternalInput").ap()
    w2_d = nc.dram_tensor("conv2_w", [4 * C, C, 3, 3], f32, kind="ExternalInput").ap()
    b2_d = nc.dram_tensor("conv2_b", [4 * C], f32, kind="ExternalInput").ap()
    w3_d = nc.dram_tensor("conv3_w", [1, C, 1, 1], f32, kind="ExternalInput").ap()
    b3_d = nc.dram_tensor("conv3_b", [1], f32, kind="ExternalInput").ap()
    o_d = nc.dram_tensor("out", [C, H, H], f32, kind="ExternalOutput").ap()

    with tile.TileContext(nc) as tc:
        with ExitStack() as ctx:
            _kernel(ctx, tc, nc, x_d, w1_d, b1_d, dkw_d, dkb_d, fcw_d, fcb_d,
                    w2_d, b2_d, w3_d, b3_d, o_d)
    nc.compile()
    return nc


def _kernel(ctx, tc, nc, x_d, w1_d, b1_d, dkw_d, dkb_d, fcw_d, fcb_d,
            w2_d, b2_d, w3_d, b3_d, o_d):
    persist = ctx.enter_context(tc.tile_pool(name="persist", bufs=1))
    xcp = ctx.enter_context(tc.tile_pool(name="xcp", bufs=1))
    hpool = ctx.enter_context(tc.tile_pool(name="h", bufs=2))
    stagep = ctx.enter_context(tc.tile_pool(name="stage", bufs=2))
    small = ctx.enter_context(tc.tile_pool(name="small", bufs=4))
    diagp = ctx.enter_context(tc.tile_pool(name="diag", bufs=2))
    outp = ctx.enter_context(tc.tile_pool(name="outb", bufs=2))
    psum = ctx.enter_context(tc.tile_pool(name="psum", bufs=4, space="PSUM"))
    psmall = ctx.enter_context(tc.tile_pool(name="psmall", bufs=2, space="PSUM"))

    # ---------------- one-time setup ----------------
    zrow = persist.tile([128, XW], f32)
    nc.vector.memset(zrow[:], 0.0)

    # conv1 weights, block-diagonal [K=128, 9, M=128] fp16
    w1stage = persist.tile([C, 9, C], f32)
    for t in range(9):
        nc.sync.dma_start(out=w1stage[:, t, :],
                          in_=_ap(w1_d, t, [[9, C], [576, C]]))
    w1t = persist.tile([128, 9, 128], f16)
    nc.vector.memset(w1t[:], 0.0)
    nc.scalar.activation(w1t[0:C, :, 0:C], w1stage[:], AF.Copy, bias=0.0, scale=1.0)
    nc.gpsimd.dma_start(out=w1t[C:128, :, C:128], in_=w1t[0:C, :, 0:C])

    i128 = persist.tile([128, 128], f32)
    make_identity(nc, i128[:])

    biases = persist.tile([128, 6], f32)
    nc.sync.dma_start(out=biases[0:C, 0:1], in_=b1_d.unsqueeze(-1))
    for i in range(NL):
        nc.sync.dma_start(out=biases[0:C, 1 + i:2 + i], in_=dkb_d[i].unsqueeze(-1))
    nc.gpsimd.dma_start(out=biases[C:128, :], in_=biases[0:C, :])

    w2dt = persist.tile([16, NL, C], f32r)
    fcwt = persist.tile([C, NL, 18], f32r)
    fcb = persist.tile([1, NL, 18], f32)
    for i in range(NL):
        nc.sync.dma_start(out=w2dt[:, i, :],
                          in_=_ap(dkw_d.bitcast(f32r), i * 1024, [[1, 16], [16, C]]))
        nc.sync.dma_start(out=fcwt[:, i, :],
                          in_=_ap(fcw_d.bitcast(f32r), i * 18 * C, [[1, C], [C, 18]]))
        nc.sync.dma_start(out=fcb[:, i, :], in_=fcb_d[i:i + 1, :])

    it = small.tile([16, 9], mybir.dt.int32, tag="it")
    by16 = persist.tile([16, 9], f32)
    bx16 = persist.tile([16, 9], f32)
    nc.gpsimd.iota(it[:], pattern=[[1, 3], [0, 3]], base=0, channel_multiplier=0)
    nc.vector.tensor_copy(by16[:], it[:])
    nc.vector.tensor_scalar_add(by16[:], by16[:], 0.5)
    nc.gpsimd.iota(it[:], pattern=[[0, 3], [1, 3]], base=0, channel_multiplier=0)
    nc.vector.tensor_copy(bx16[:], it[:])
    nc.vector.tensor_scalar_add(bx16[:], bx16[:], 0.5)
    itp = small.tile([16, 1], mybir.dt.int32, tag="itp")
    idx16 = small.tile([16, 1], f32, tag="idx16")
    ky16 = persist.tile([16, 1], f32)
    kx16 = persist.tile([16, 1], f32)
    nc.gpsimd.iota(itp[:], pattern=[[0, 1]], base=0, channel_multiplier=1)
    nc.vector.tensor_copy(idx16[:], itp[:])
    st16 = small.tile([1, 16], mybir.dt.int32, tag="st16")
    nc.gpsimd.iota(st16[:], pattern=[[1, 4], [0, 4]], base=0, channel_multiplier=0)
    stf = small.tile([1, 16], f32, tag="stf")
    nc.vector.tensor_copy(stf[:], st16[:])
    nc.gpsimd.dma_start(out=ky16[:], in_=_ap(stf, 0, [[1, 16], [16, 1]]))
    nc.vector.scalar_tensor_tensor(kx16[:], ky16[:], -4.0, idx16[:], ALU.mult, ALU.add)

    # fused conv2/conv3: W2f[ci, t, j] = sum_c conv2_w[4c+j, ci, t] * conv3_w[c]
    w3sb = persist.tile([C, 4], f32r)
    nc.gpsimd.dma_start(out=w3sb[:].unsqueeze(-1),
                        in_=_ap(w3_d.bitcast(f32r), 0, [[1, C], [0, 4], [1, 1]]))
    c2wj = persist.tile([C, 4, 576], f32r)
    for j in range(4):
        nc.sync.dma_start(out=c2wj[:, j, :],
                          in_=_ap(w2_d.bitcast(f32r), j * 576, [[4 * 576, C], [1, 576]]))
    w2f_ps = psmall.tile([C, 144], f32, tag="sp")
    for t in range(9):
        for j in range(4):
            lhsT = _ap(c2wj, j * 576 + t, [c2wj.ap[0], [9, C]])
            k = (t * 4 + j) * 4
            nc.tensor.matmul(w2f_ps[:, k:k + 4], lhsT, w3sb[:],
                             start=True, stop=True)
    # block-diagonal [K=128, 9, M=8] fp16
    w2ft = persist.tile([128, 9, 8], f16)
    nc.vector.memset(w2ft[:], 0.0)
    nc.scalar.activation(w2ft[0:C, :, 0:4],
                         _ap(w2f_ps, 0, [w2f_ps.ap[0], [16, 9], [4, 4]]),
                         AF.Copy, bias=0.0, scale=1.0)
    nc.gpsimd.dma_start(out=w2ft[C:128, :, 4:8], in_=w2ft[0:C, :, 0:4])
    c2bj = persist.tile([C, 4], f32r)
    nc.sync.dma_start(out=c2bj[:], in_=_ap(b2_d.bitcast(f32r), 0, [[4, C], [1, 4]]))
    b2f_ps = psmall.tile([4, 4], f32, tag="sp")
    nc.tensor.matmul(b2f_ps[:], c2bj[:], w3sb[:], start=True, stop=True)
    b3b8 = small.tile([8, 1], f32, tag="b3b")
    nc.gpsimd.dma_start(out=b3b8[:], in_=_ap(b3_d, 0, [[0, 8], [1, 1]]))
    b2f8 = persist.tile([8, 1], f32)
    nc.scalar.activation(b2f8[0:4], b2f_ps[:, 0:1], AF.Copy, bias=0.0, scale=1.0)
    nc.gpsimd.dma_start(out=b2f8[4:8], in_=b2f8[0:4])
    nc.vector.tensor_add(b2f8[:], b2f8[:], b3b8[:])

    # att replication selectors: sel8[p, j, q] = 1 iff p == 4*(q//64) + j
    sel8f = persist.tile([8, 4, 128], f32)
    nc.gpsimd.memset(sel8f[:], 0.0)
    for j in range(4):
        for g in range(2):
            nc.gpsimd.affine_select(out=sel8f[:, j, 64 * g:64 * (g + 1)],
                                    in_=sel8f[:, j, 64 * g:64 * (g + 1)],
                                    pattern=[[0, 64]], compare_op=ALU.not_equal,
                                    fill=1.0, base=-(4 * g + j),
                                    channel_multiplier=1)
    sel8 = persist.tile([8, 4, 128], f16)
    nc.scalar.activation(sel8[:], sel8f[:], AF.Copy, bias=0.0, scale=1.0)

    ones116f = persist.tile([1, 16], f32)
    nc.vector.memset(ones116f, 1.0)
    ones116 = persist.tile([1, 16], f32r)
    nc.scalar.activation(ones116[:], ones116f[:], AF.Copy, bias=0.0, scale=1.0)

    att8 = persist.tile([8, RB, HH], f16)

    # ---------------- x cache (fp16, both halves, gutters + halo) ----------------
    xc = xcp.tile([128, XSL, XW], f16)
    zc = _ap(zrow, 0, [zrow.ap[0], [1, XSL], [1, 1]])
    nc.scalar.activation(xc[:, :, 0:1], zc, AF.Copy, bias=0.0, scale=1.0)
    nc.scalar.activation(xc[:, :, XW - 1:XW], zc, AF.Copy, bias=0.0, scale=1.0)
    nc.scalar.activation(xc[0:C, 0, :], zrow[0:C, :], AF.Copy, bias=0.0, scale=1.0)
    nc.scalar.activation(xc[:, XSL - 1, :], zrow[:, :], AF.Copy, bias=0.0, scale=1.0)

    # load chunk 7 first: bottom half's top halo (x row 127) comes from it
    for ci, k in enumerate([7, 0, 1, 2, 3, 4, 5, 6]):
        r0 = CH * k
        st = stagep.tile([128, CH, H], f32, tag="xst")
        nc.sync.dma_start(out=st[0:C], in_=x_d[:, r0:r0 + CH, :])
        nc.sync.dma_start(out=st[C:128], in_=x_d[:, 128 + r0:128 + r0 + CH, :])
        dst = xc[:, 1 + r0:1 + r0 + CH, 1:H + 1]
        if ci % 2 == 0:
            nc.scalar.activation(dst, st[:], AF.Copy, bias=0.0, scale=1.0)
        else:
            nc.vector.tensor_copy(dst, st[:])
        if k == 7:
            nc.gpsimd.dma_start(out=xc[C:128, 0, :], in_=xc[0:C, 128, :])

    # ---------------- h tensors ----------------
    def new_h():
        h = hpool.tile([128, SLOTS, WCOL], f16, tag="h")
        nc.scalar.activation(h[0:C, 0, :], zrow[0:C, 0:WCOL], AF.Copy, bias=0.0, scale=1.0)
        nc.scalar.activation(h[C:128, SLOTS - 1, :], zrow[C:128, 0:WCOL], AF.Copy, bias=0.0, scale=1.0)
        zch = _ap(zrow, 0, [zrow.ap[0], [1, SLOTS], [1, 1]])
        nc.scalar.activation(h[:, :, 0:1], zch, AF.Copy, bias=0.0, scale=1.0)
        nc.scalar.activation(h[:, :, WCOL - 1:WCOL], zch, AF.Copy, bias=0.0, scale=1.0)
        return h

    def halo_fix(h):
        nc.gpsimd.dma_start(out=h[C:128, 0, :], in_=h[0:C, RB, :])
        nc.gpsimd.dma_start(out=h[0:C, SLOTS - 1, :], in_=h[C:128, 1, :])

    # ---------------- conv1 (stride 2) ----------------
    h1 = new_h()
    pp1 = small.tile([128, NB], f32, tag="pp")
    for b in range(NB):
        ps = psum.tile([128, BR, HH], f32, tag="cps")
        for t in range(9):
            dy, dx = TAPS[t]
            rhs = xc[:, 1 + 2 * BR * b + dy:1 + 2 * BR * b + dy + 2 * BR - 1:2,
                     1 + dx:1 + dx + 2 * HH - 1:2]
            nc.tensor.matmul(ps[:], w1t[:, t, :], rhs,
                             start=(t == 0), stop=(t == 8))
        s0 = 1 + BR * b
        nc.scalar.activation(h1[:, s0:s0 + BR, 1:HH + 1], ps[:],
                             AF.Relu, bias=biases[:, 0:1], scale=1.0,
                             accum_out=pp1[:, b:b + 1])
    halo_fix(h1)

    # ---------------- dkc layers ----------------
    h_cur, pp_cur = h1, pp1
    for li in range(NL):
        red = small.tile([128, 1], f32, tag="red")
        nc.vector.tensor_reduce(out=red[:], in_=pp_cur[:], axis=AX.X, op=ALU.add)
        tmp64 = small.tile([C, 1], f32, tag="t64")
        nc.gpsimd.dma_start(out=tmp64[:], in_=red[C:128, :])
        featf = small.tile([C, 1], f32, tag="featf")
        nc.vector.tensor_add(featf[:], red[0:C, :], tmp64[:])
        feat = small.tile([C, 1], f32r, tag="feat")
        nc.scalar.activation(feat[:], featf[:], AF.Copy, bias=0.0, scale=1.0 / 16384.0)
        offp = psmall.tile([1, 18], f32, tag="sp")
        nc.tensor.matmul(offp[:], feat[:], fcwt[:, li, :], start=True, stop=True)
        offf = small.tile([1, 18], f32, tag="offf")
        nc.vector.tensor_add(offf[:], offp[:], fcb[:, li, :])
        off = small.tile([1, 18], f32r, tag="off")
        nc.scalar.activation(off[:], offf[:], AF.Copy, bias=0.0, scale=1.0)
        offr_ps = psmall.tile([16, 18], f32, tag="sp")
        nc.tensor.matmul(offr_ps[:], ones116[:], off[:], start=True, stop=True)
        phiy = small.tile([16, 9], f32, tag="phiy")
        phix = small.tile([16, 9], f32, tag="phix")
        kintf = small.tile([16, 10], f32, tag="kintf")
        kint = small.tile([16, 10], f32r, tag="kint")
        nc.vector.memset(kintf[:, 9:10], 0.0)
        nc.vector.tensor_add(phiy[:], offr_ps[:, 0:9], by16[:])
        nc.vector.tensor_scalar(phiy[:], phiy[:], ky16[:], None, ALU.subtract)
        nc.scalar.activation(phiy[:], phiy[:], AF.Abs, bias=0.0, scale=1.0)
        nc.scalar.activation(phiy[:], phiy[:], AF.Relu, bias=1.0, scale=-1.0)
        nc.vector.tensor_add(phix[:], offr_ps[:, 9:18], bx16[:])
        nc.vector.tensor_scalar(phix[:], phix[:], kx16[:], None, ALU.subtract)
        nc.scalar.activation(phix[:], phix[:], AF.Abs, bias=0.0, scale=1.0)
        nc.scalar.activation(phix[:], phix[:], AF.Relu, bias=1.0, scale=-1.0)
        nc.vector.tensor_tensor(kintf[:, 0:9], phiy[:], phix[:], ALU.mult)
        nc.scalar.activation(kint[:], kintf[:], AF.Copy, bias=0.0, scale=1.0)
        samp_ps = psmall.tile([C, 10], f32, tag="sp")
        nc.tensor.matmul(samp_ps[:], w2dt[:, li, :], kint[:], start=True, stop=True)
        samp = small.tile([128, 9], f32, tag="samp")
        nc.scalar.activation(samp[0:C, :], samp_ps[:, 0:9], AF.Copy, bias=0.0, scale=1.0)
        nc.gpsimd.dma_start(out=samp[C:128, :], in_=samp[0:C, :])
        diag = diagp.tile([128, 9, 128], f16, tag="diag")
        for t in range(9):
            nc.vector.tensor_scalar(diag[:, t, :], i128[:], samp[:, t:t + 1],
                                    None, ALU.mult)

        h_nxt = new_h()
        if li < NL - 1:
            pp_nxt = small.tile([128, NB], f32, tag="pp")
        else:
            pp_nxt = None
        for b in range(NB):
            s0 = 1 + BR * b
            ps = psum.tile([128, BR, HH], f32, tag="cps")
            for t in range(9):
                dy, dx = TAPS[t]
                rhs = h_cur[:, s0 + dy:s0 + dy + BR, 1 + dx:1 + dx + HH]
                nc.tensor.matmul(ps[:], diag[:, t, :], rhs,
                                 start=(t == 0), stop=(t == 8))
            if pp_nxt is not None:
                nc.scalar.activation(h_nxt[:, s0:s0 + BR, 1:HH + 1], ps[:],
                                     AF.Relu, bias=biases[:, 1 + li:2 + li],
                                     scale=1.0, accum_out=pp_nxt[:, b:b + 1])
            else:
                nc.scalar.activation(h_nxt[:, s0:s0 + BR, 1:HH + 1], ps[:],
                                     AF.Relu, bias=biases[:, 1 + li:2 + li],
                                     scale=1.0)
        halo_fix(h_nxt)
        h_cur, pp_cur = h_nxt, pp_nxt

    # ---------------- fused conv2' -> sigmoid att; selector replicate; out ----------------
    for b in range(NB):
        s0 = 1 + BR * b
        ps2 = psum.tile([128, BR, HH], f32, tag="cps")
        for t in range(9):
            dy, dx = TAPS[t]
            rhs = h_cur[:, s0 + dy:s0 + dy + BR, 1 + dx:1 + dx + HH]
            nc.tensor.matmul(ps2[0:8, :, :], w2ft[:, t, :], rhs,
                             start=(t == 0), stop=(t == 8))
        nc.scalar.activation(att8[:, BR * b:BR * b + BR, :], ps2[0:8, :, :],
                             AF.Sigmoid, bias=b2f8[:], scale=1.0)
        for fb in (2 * b, 2 * b + 1):
            ob = outp.tile([128, BR, H], f32, tag="ob")
            for j in range(4):
                dy, dx = j // 2, j % 2
                pa = psum.tile([128, BR, HH], f32, tag="cps")
                nc.tensor.matmul(pa[:, 0:YY, :], sel8[:, j, :],
                                 att8[0:8, YY * fb:YY * fb + YY, :],
                                 start=True, stop=True, skip_group_check=True)
                xv = xc[:, 1 + BR * fb + dy:1 + BR * fb + dy + BR - 1:2,
                        1 + dx:1 + dx + H - 1:2]
                ov = _ap(ob, dy * H + dx, [ob.ap[0], [2 * H, YY], [2, HH]])
                nc.vector.tensor_tensor(ov, xv, pa[:, 0:YY, :], ALU.mult)
            y0 = BR * fb
            nc.sync.dma_start(out=o_d[:, y0:y0 + BR, :], in_=ob[0:C])
            nc.sync.dma_start(out=o_d[:, 128 + y0:128 + y0 + BR, :], in_=ob[C:128])


_NC_CACHE = {}


def kernel(**inputs):
    if "nc" not in _NC_CACHE:
        _NC_CACHE["nc"] = build_nc()
    nc = _NC_CACHE["nc"]
    names = ["conv1_w", "conv1_b", "dkc_w", "dkc_b", "dkc_fc_w", "dkc_fc_b",
             "conv2_w", "conv2_b", "conv3_w", "conv3_b"]
    shared = {n: np.ascontiguousarray(np.asarray(inputs[n], dtype=np.float32))
              for n in names}
    x = np.asarray(inputs["x"], dtype=np.float32)
    in_maps = [dict(shared, x=np.ascontiguousarray(x[i])) for i in range(8)]
    r = run_bass_kernel_spmd(nc, in_maps, list(range(8)))
    _NC_CACHE["last_result"] = r
    return np.stack([r.results[i]["out"] for i in range(8)]).astype(np.float32)


# revision 11
# speedup vs baseline: 1.5029x; 1.0676x over previous
"""TRN2 Bass/Tile kernel: deformable-kernel spatial attention (dense_cnn).

Per-core (pure data parallel, batch 8 over 8 cores):
  x cached in SBUF as fp16 with column-parity-split layout (single HBM
  read, contiguous moving-operand reads for the stride-2 conv1 and the
  final attention multiply). Spatial halves packed on partitions 0:64 /
  64:128 so every matmul runs K=128 with block-diagonal weights.
  h1 = relu(conv1(x))     3x3 stride-2 64->64, 9 tap-matmuls, 4-row bands
  5x dkc:                 global-pool -> fc offsets -> hat-function
                          resample of the 4x4 scope kernel -> depthwise
                          3x3 as 9 diagonal matmuls accumulated in PSUM
  conv2+pixel_shuffle+conv3 fused into a 64->4 channel 3x3 conv
  att replicated across channels with K=8 selector matmuls; final
  out = x * att on DVE from the fp16 x cache.
"""

import numpy as np

import concourse.bass as bass
import concourse.mybir as mybir
import concourse.tile as tile
from concourse import bacc
from concourse.bass_utils import run_bass_kernel_spmd
from concourse.masks import make_identity
from contextlib import ExitStack

f32 = mybir.dt.float32
f32r = mybir.dt.float32r
f16 = mybir.dt.float16
AF = mybir.ActivationFunctionType
ALU = mybir.AluOpType
AX = mybir.AxisListType

C = 64
H = 256
HH = 128
RB = 64          # interior feature rows per half
SLOTS = RB + 2   # + top/bottom halo row
WCOL = HH + 2    # zero gutter columns at 0 and 129
XSL = 130        # x-cache slots per half: halo, 128 rows, (unused)
XW = 257         # x-cache cols: [zero | odd cols 1..128 | even cols 129..256]
NL = 5
BR = 4           # output rows per band (moving operand max 512 elements)
NB = RB // BR    # bands per conv phase
XCH = 8          # x rows per load chunk (per half)
NCH = 128 // XCH

TAPS = [(t // 3 - 1, t % 3 - 1) for t in range(9)]  # t = 3*ty+tx -> (dy, dx)
# x-cache column base for a conv1 tap reading input col 2*xo+dx
XCB = {-1: 0, 0: 129, 1: 1}


def _ap(a, extra_off, dims):
    return bass.AP(tensor=a.tensor, offset=a.offset + extra_off, ap=dims)


def build_nc():
    nc = bacc.Bacc("TRN2", target_bir_lowering=False, debug=False)
    x_d = nc.dram_tensor("x", [C, H, H], f32, kind="ExternalInput").ap()
    w1_d = nc.dram_tensor("conv1_w", [C, C, 3, 3], f32, kind="ExternalInput").ap()
    b1_d = nc.dram_tensor("conv1_b", [C], f32, kind="ExternalInput").ap()
    dkw_d = nc.dram_tensor("dkc_w", [NL, C, 1, 4, 4], f32, kind="ExternalInput").ap()
    dkb_d = nc.dram_tensor("dkc_b", [NL, C], f32, kind="ExternalInput").ap()
    fcw_d = nc.dram_tensor("dkc_fc_w", [NL, 18, C], f32, kind="ExternalInput").ap()
    fcb_d = nc.dram_tensor("dkc_fc_b", [NL, 18], f32, kind="ExternalInput").ap()
    w2_d = nc.dram_tensor("conv2_w", [4 * C, C, 3, 3], f32, kind="ExternalInput").ap()
    b2_d = nc.dram_tensor("conv2_b", [4 * C], f32, kind="ExternalInput").ap()
    w3_d = nc.dram_tensor("conv3_w", [1, C, 1, 1], f32, kind="ExternalInput").ap()
    b3_d = nc.dram_tensor("conv3_b", [1], f32, kind="ExternalInput").ap()
    o_d = nc.dram_tensor("out", [C, H, H], f32, kind="ExternalOutput").ap()

    with tile.TileContext(nc) as tc:
        with ExitStack() as ctx:
            _kernel(ctx, tc, nc, x_d, w1_d, b1_d, dkw_d, dkb_d, fcw_d, fcb_d,
                    w2_d, b2_d, w3_d, b3_d, o_d)
    nc.compile()
    return nc


def _kernel(ctx, tc, nc, x_d, w1_d, b1_d, dkw_d, dkb_d, fcw_d, fcb_d,
            w2_d, b2_d, w3_d, b3_d, o_d):
    persist = ctx.enter_context(tc.tile_pool(name="persist", bufs=1))
    xcp = ctx.enter_context(tc.tile_pool(name="xcp", bufs=1))
    hpool = ctx.enter_context(tc.tile_pool(name="h", bufs=2))
    stagep = ctx.enter_context(tc.tile_pool(name="stage", bufs=3))
    small = ctx.enter_context(tc.tile_pool(name="small", bufs=4))
    diagp = ctx.enter_context(tc.tile_pool(name="diag", bufs=2))
    outp = ctx.enter_context(tc.tile_pool(name="outb", bufs=2))
    psum = ctx.enter_context(tc.tile_pool(name="psum", bufs=6, space="PSUM"))
    psmall = ctx.enter_context(tc.tile_pool(name="psmall", bufs=2, space="PSUM"))

    # ---------------- one-time setup (weight staging DMAs on gpsimd) ----------------
    zrow = persist.tile([128, XW], f32)
    nc.vector.memset(zrow[:], 0.0)

    # conv1 weights, block-diagonal [K=128, 9, M=128] fp16
    w1stage = persist.tile([C, 9, C], f32)
    for t in range(9):
        nc.gpsimd.dma_start(out=w1stage[:, t, :],
                            in_=_ap(w1_d, t, [[9, C], [576, C]]))
    w1t = persist.tile([128, 9, 128], f16)
    nc.vector.memset(w1t[:], 0.0)
    nc.scalar.activation(w1t[0:C, :, 0:C], w1stage[:], AF.Copy, bias=0.0, scale=1.0)
    nc.gpsimd.dma_start(out=w1t[C:128, :, C:128], in_=w1t[0:C, :, 0:C])

    i128 = persist.tile([128, 128], f32)
    make_identity(nc, i128[:])

    biases = persist.tile([128, 6], f32)
    nc.gpsimd.dma_start(out=biases[0:C, 0:1], in_=b1_d.unsqueeze(-1))
    for i in range(NL):
        nc.gpsimd.dma_start(out=biases[0:C, 1 + i:2 + i], in_=dkb_d[i].unsqueeze(-1))
    nc.gpsimd.dma_start(out=biases[C:128, :], in_=biases[0:C, :])

    # dkc scope weights replicated across both halves: w2dt2[s, li, q] = w2d[q%64, s]
    w2dt2 = persist.tile([16, NL, 128], f32r)
    for i in range(NL):
        src = _ap(dkw_d.bitcast(f32r), i * 1024, [[1, 16], [16, C]])
        nc.gpsimd.dma_start(out=w2dt2[:, i, 0:C], in_=src)
        nc.gpsimd.dma_start(out=w2dt2[:, i, C:128], in_=src)
    # fc weights replicated + pre-scaled by 1/16384 (global mean folded in)
    fcwtf = persist.tile([128, NL, 18], f32)
    for i in range(NL):
        src = _ap(fcw_d, i * 18 * C, [[1, C], [C, 18]])
        nc.gpsimd.dma_start(out=fcwtf[0:C, i, :], in_=src)
        nc.gpsimd.dma_start(out=fcwtf[C:128, i, :], in_=src)
    fcwt2 = persist.tile([128, NL, 18], f32r)
    nc.vector.tensor_scalar(fcwt2[:], fcwtf[:], 1.0 / 16384.0, None, ALU.mult)

    # hat-function grid constants, with fc bias folded in:
    # bk18[s, li, :] = [by|bx](t) + fcb[li] - [ky|kx](s)
    it = small.tile([16, 18], mybir.dt.int32, tag="it")
    b18 = persist.tile([16, 18], f32)
    nc.gpsimd.iota(it[:, 0:9], pattern=[[1, 3], [0, 3]], base=0, channel_multiplier=0)
    nc.gpsimd.iota(it[:, 9:18], pattern=[[0, 3], [1, 3]], base=0, channel_multiplier=0)
    nc.vector.tensor_copy(b18[:], it[:])
    nc.vector.tensor_scalar_add(b18[:], b18[:], 0.5)
    itp = small.tile([16, 1], mybir.dt.int32, tag="itp")
    idx16 = small.tile([16, 1], f32, tag="idx16")
    ky16 = persist.tile([16, 1], f32)
    kx16 = persist.tile([16, 1], f32)
    nc.gpsimd.iota(itp[:], pattern=[[0, 1]], base=0, channel_multiplier=1)
    nc.vector.tensor_copy(idx16[:], itp[:])
    st16 = small.tile([1, 16], mybir.dt.int32, tag="st16")
    nc.gpsimd.iota(st16[:], pattern=[[1, 4], [0, 4]], base=0, channel_multiplier=0)
    stf = small.tile([1, 16], f32, tag="stf")
    nc.vector.tensor_copy(stf[:], st16[:])
    nc.gpsimd.dma_start(out=ky16[:], in_=_ap(stf, 0, [[1, 16], [16, 1]]))
    nc.vector.scalar_tensor_tensor(kx16[:], ky16[:], -4.0, idx16[:], ALU.mult, ALU.add)
    ones1618 = persist.tile([16, 18], f32)
    nc.vector.memset(ones1618[:], 1.0)
    k18 = persist.tile([16, 18], f32)
    nc.vector.tensor_scalar(k18[:, 0:9], ones1618[:, 0:9], ky16[:], None, ALU.mult)
    nc.vector.tensor_scalar(k18[:, 9:18], ones1618[:, 9:18], kx16[:], None, ALU.mult)
    fcb16 = persist.tile([16, NL, 18], f32)
    nc.gpsimd.dma_start(out=fcb16[:], in_=_ap(fcb_d, 0, [[0, 16], [18, NL], [1, 18]]))
    bk18 = persist.tile([16, NL, 18], f32)
    bmk = persist.tile([16, 18], f32)
    nc.vector.tensor_tensor(bmk[:], b18[:], k18[:], ALU.subtract)
    for i in range(NL):
        nc.vector.tensor_tensor(bk18[:, i, :], bmk[:], fcb16[:, i, :], ALU.add)
    kint = persist.tile([16, 10], f32r)
    nc.vector.tensor_copy(kint[:], zrow[0:16, 0:10])
    ones116f = persist.tile([1, 16], f32)
    nc.vector.memset(ones116f, 1.0)
    ones116 = persist.tile([1, 16], f32r)
    nc.vector.tensor_copy(ones116[:], ones116f[:])

    # fused conv2/conv3: W2f[ci, t, j] = sum_c conv2_w[4c+j, ci, t] * conv3_w[c]
    w3sb = persist.tile([C, 4], f32r)
    nc.gpsimd.dma_start(out=w3sb[:].unsqueeze(-1),
                        in_=_ap(w3_d.bitcast(f32r), 0, [[1, C], [0, 4], [1, 1]]))
    c2wj = persist.tile([C, 4, 576], f32r)
    for j in range(4):
        nc.gpsimd.dma_start(out=c2wj[:, j, :],
                            in_=_ap(w2_d.bitcast(f32r), j * 576, [[4 * 576, C], [1, 576]]))
    w2f_ps = psmall.tile([C, 144], f32, tag="sp")
    for t in range(9):
        for j in range(4):
            lhsT = _ap(c2wj, j * 576 + t, [c2wj.ap[0], [9, C]])
            k = (t * 4 + j) * 4
            nc.tensor.matmul(w2f_ps[:, k:k + 4], lhsT, w3sb[:],
                             start=True, stop=True)
    # block-diagonal [K=128, 9, M=8] fp16
    w2ft = persist.tile([128, 9, 8], f16)
    nc.vector.memset(w2ft[:], 0.0)
    nc.scalar.activation(w2ft[0:C, :, 0:4],
                         _ap(w2f_ps, 0, [w2f_ps.ap[0], [16, 9], [4, 4]]),
                         AF.Copy, bias=0.0, scale=1.0)
    nc.gpsimd.dma_start(out=w2ft[C:128, :, 4:8], in_=w2ft[0:C, :, 0:4])
    c2bj = persist.tile([C, 4], f32r)
    nc.gpsimd.dma_start(out=c2bj[:], in_=_ap(b2_d.bitcast(f32r), 0, [[4, C], [1, 4]]))
    b2f_ps = psmall.tile([4, 4], f32, tag="sp")
    nc.tensor.matmul(b2f_ps[:], c2bj[:], w3sb[:], start=True, stop=True)
    b3b8 = small.tile([8, 1], f32, tag="b3b")
    nc.gpsimd.dma_start(out=b3b8[:], in_=_ap(b3_d, 0, [[0, 8], [1, 1]]))
    b2f8 = persist.tile([8, 1], f32)
    nc.scalar.activation(b2f8[0:4], b2f_ps[:, 0:1], AF.Copy, bias=0.0, scale=1.0)
    nc.gpsimd.dma_start(out=b2f8[4:8], in_=b2f8[0:4])
    nc.vector.tensor_add(b2f8[:], b2f8[:], b3b8[:])

    # att replication selectors: sel8[p, j, q] = 1 iff p == 4*(q//64) + j
    sel8f = persist.tile([8, 4, 128], f32)
    nc.gpsimd.memset(sel8f[:], 0.0)
    for j in range(4):
        for g in range(2):
            nc.gpsimd.affine_select(out=sel8f[:, j, 64 * g:64 * (g + 1)],
                                    in_=sel8f[:, j, 64 * g:64 * (g + 1)],
                                    pattern=[[0, 64]], compare_op=ALU.not_equal,
                                    fill=1.0, base=-(4 * g + j),
                                    channel_multiplier=1)
    sel8 = persist.tile([8, 4, 128], f16)
    nc.scalar.activation(sel8[:], sel8f[:], AF.Copy, bias=0.0, scale=1.0)

    att8 = persist.tile([8, RB, HH], f16)

    # ---------------- x cache: fp16, column-parity split, zero gutters ----------------
    xc = xcp.tile([128, XSL, XW], f16)
    zc = _ap(zrow, 0, [zrow.ap[0], [1, XSL], [1, 1]])
    nc.scalar.activation(xc[:, :, 0:1], zc, AF.Copy, bias=0.0, scale=1.0)
    nc.scalar.activation(xc[0:C, 0, :], zrow[0:C, 0:XW], AF.Copy, bias=0.0, scale=1.0)
    nc.scalar.activation(xc[:, XSL - 1, :], zrow[:, 0:XW], AF.Copy, bias=0.0, scale=1.0)

    # load chunk 15 first: bottom half's top halo (x row 127) comes from it
    for ci, k in enumerate([NCH - 1] + list(range(NCH - 1))):
        r0 = XCH * k
        st = stagep.tile([128, XCH, H], f32, tag="xst")
        nc.sync.dma_start(out=st[0:C], in_=x_d[:, r0:r0 + XCH, :])
        nc.gpsimd.dma_start(out=st[C:128], in_=x_d[:, 128 + r0:128 + r0 + XCH, :])
        # odd cols -> 1..128, even cols -> 129..256 (engines split per chunk)
        if ci % 2 == 0:
            nc.vector.tensor_copy(xc[:, 1 + r0:1 + r0 + XCH, 1:129], st[:, :, 1:H:2])
            nc.scalar.activation(xc[:, 1 + r0:1 + r0 + XCH, 129:257], st[:, :, 0:H:2],
                                 AF.Copy, bias=0.0, scale=1.0)
        else:
            nc.scalar.activation(xc[:, 1 + r0:1 + r0 + XCH, 1:129], st[:, :, 1:H:2],
                                 AF.Copy, bias=0.0, scale=1.0)
            nc.vector.tensor_copy(xc[:, 1 + r0:1 + r0 + XCH, 129:257], st[:, :, 0:H:2])
        if k == NCH - 1:
            nc.gpsimd.dma_start(out=xc[C:128, 0, :], in_=xc[0:C, 128, :])

    # ---------------- h tensors ----------------
    def new_h():
        h = hpool.tile([128, SLOTS, WCOL], f16, tag="h")
        nc.scalar.activation(h[0:C, 0, :], zrow[0:C, 0:WCOL], AF.Copy, bias=0.0, scale=1.0)
        nc.scalar.activation(h[C:128, SLOTS - 1, :], zrow[C:128, 0:WCOL], AF.Copy, bias=0.0, scale=1.0)
        zch = _ap(zrow, 0, [zrow.ap[0], [1, SLOTS], [1, 1]])
        nc.scalar.activation(h[:, :, 0:1], zch, AF.Copy, bias=0.0, scale=1.0)
        nc.scalar.activation(h[:, :, WCOL - 1:WCOL], zch, AF.Copy, bias=0.0, scale=1.0)
        return h

    def halo_fix(h):
        nc.gpsimd.dma_start(out=h[C:128, 0, :], in_=h[0:C, RB, :])
        nc.gpsimd.dma_start(out=h[0:C, SLOTS - 1, :], in_=h[C:128, 1, :])

    # ---------------- conv1 (stride 2) ----------------
    h1 = new_h()
    pp1 = small.tile([128, NB], f32, tag="pp")
    for b in range(NB):
        ps = psum.tile([128, BR, HH], f32, tag="cps")
        for t in range(9):
            dy, dx = TAPS[t]
            cb = XCB[dx]
            rhs = xc[:, 1 + 2 * BR * b + dy:1 + 2 * BR * b + dy + 2 * BR - 1:2,
                     cb:cb + HH]
            nc.tensor.matmul(ps[:], w1t[:, t, :], rhs,
                             start=(t == 0), stop=(t == 8))
        s0 = 1 + BR * b
        nc.scalar.activation(h1[:, s0:s0 + BR, 1:HH + 1], ps[:],
                             AF.Relu, bias=biases[:, 0:1], scale=1.0,
                             accum_out=pp1[:, b:b + 1])
    halo_fix(h1)

    # ---------------- dkc layers ----------------
    h_cur, pp_cur = h1, pp1
    for li in range(NL):
        redr = small.tile([128, 1], f32r, tag="redr")
        with nc.allow_low_precision(reason="f32r is fp32 bits (PE fast-load fmt)"):
            nc.vector.tensor_reduce(out=redr[:], in_=pp_cur[:], axis=AX.X, op=ALU.add)
        offp = psmall.tile([1, 18], f32, tag="sp")
        nc.tensor.matmul(offp[:], redr[:], fcwt2[:, li, :], start=True, stop=True)
        off = small.tile([1, 18], f32r, tag="off")
        nc.vector.tensor_copy(off[:], offp[:])
        offr_ps = psmall.tile([16, 18], f32, tag="sp")
        nc.tensor.matmul(offr_ps[:], ones116[:], off[:], start=True, stop=True)
        # phi = relu(1 - |offp + b - k|) = relu(min(1 - d, 1 + d))
        phi = small.tile([16, 18], f32, tag="phi")
        phia = small.tile([16, 18], f32, tag="phia")
        nc.vector.tensor_tensor(phi[:], offr_ps[:], bk18[:, li, :], ALU.add)
        nc.vector.scalar_tensor_tensor(phia[:], phi[:], 1.0, ones1618[:],
                                       ALU.mult, ALU.add)
        nc.vector.scalar_tensor_tensor(phi[:], phi[:], -1.0, ones1618[:],
                                       ALU.mult, ALU.add)
        nc.vector.tensor_tensor(phi[:], phi[:], phia[:], ALU.min)
        nc.vector.tensor_scalar(phi[:], phi[:], 0.0, None, ALU.max)
        nc.vector.tensor_tensor(kint[:, 0:9], phi[:, 0:9], phi[:, 9:18], ALU.mult)
        samp_ps = psmall.tile([128, 10], f32, tag="sp")
        nc.tensor.matmul(samp_ps[:], w2dt2[:, li, :], kint[:], start=True, stop=True)
        sampf = small.tile([128, 9], f32, tag="samp")
        nc.vector.tensor_copy(sampf[:], samp_ps[:, 0:9])
        diag = diagp.tile([128, 9, 128], f16, tag="diag")
        for t in range(9):
            if t % 2 == 0:
                nc.vector.tensor_scalar(diag[:, t, :], i128[:], sampf[:, t:t + 1],
                                        None, ALU.mult)
            else:
                nc.scalar.activation(diag[:, t, :], i128[:], AF.Copy,
                                     bias=0.0, scale=sampf[:, t:t + 1])

        h_nxt = new_h()
        if li < NL - 1:
            pp_nxt = small.tile([128, NB], f32, tag="pp")
        else:
            pp_nxt = None
        for b in range(NB):
            s0 = 1 + BR * b
            ps = psum.tile([128, BR, HH], f32, tag="cps")
            for t in range(9):
                dy, dx = TAPS[t]
                rhs = h_cur[:, s0 + dy:s0 + dy + BR, 1 + dx:1 + dx + HH]
                nc.tensor.matmul(ps[:], diag[:, t, :], rhs,
                                 start=(t == 0), stop=(t == 8))
            if pp_nxt is not None:
                nc.scalar.activation(h_nxt[:, s0:s0 + BR, 1:HH + 1], ps[:],
                                     AF.Relu, bias=biases[:, 1 + li:2 + li],
                                     scale=1.0, accum_out=pp_nxt[:, b:b + 1])
            else:
                nc.scalar.activation(h_nxt[:, s0:s0 + BR, 1:HH + 1], ps[:],
                                     AF.Relu, bias=biases[:, 1 + li:2 + li],
                                     scale=1.0)
        halo_fix(h_nxt)
        h_cur, pp_cur = h_nxt, pp_nxt

    # ---------------- fused conv2' -> sigmoid att; selector replicate; out ----------------
    # conv2' band b covers att rows 4b..4b+3 == output rows 8b..8b+7
    for b in range(NB):
        s0 = 1 + BR * b
        ps2 = psum.tile([128, BR, HH], f32, tag="cps")
        for t in range(9):
            dy, dx = TAPS[t]
            rhs = h_cur[:, s0 + dy:s0 + dy + BR, 1 + dx:1 + dx + HH]
            nc.tensor.matmul(ps2[0:8, :, :], w2ft[:, t, :], rhs,
                             start=(t == 0), stop=(t == 8))
        nc.scalar.activation(att8[:, BR * b:BR * b + BR, :], ps2[0:8, :, :],
                             AF.Sigmoid, bias=b2f8[:], scale=1.0)
        ob = outp.tile([128, 2 * BR, H], f32, tag="ob")
        for j in range(4):
            dy, dx = j // 2, j % 2
            pa = psum.tile([128, BR, HH], f32, tag="cps")
            nc.tensor.matmul(pa[:], sel8[:, j, :],
                             att8[0:8, BR * b:BR * b + BR, :],
                             start=True, stop=True, skip_group_check=True)
            cb = 129 if dx == 0 else 1
            xv = xc[:, 1 + 2 * BR * b + dy:1 + 2 * BR * b + dy + 2 * BR - 1:2,
                    cb:cb + HH]
            ov = _ap(ob, dy * H + dx, [ob.ap[0], [2 * H, BR], [2, HH]])
            nc.vector.tensor_tensor(ov, xv, pa[:], ALU.mult)
        y0 = 2 * BR * b
        nc.sync.dma_start(out=o_d[:, y0:y0 + 2 * BR, :], in_=ob[0:C])
        nc.gpsimd.dma_start(out=o_d[:, 128 + y0:128 + y0 + 2 * BR, :], in_=ob[C:128])


_NC_CACHE = {}


def kernel(**inputs):
    if "nc" not in _NC_CACHE:
        _NC_CACHE["nc"] = build_nc()
    nc = _NC_CACHE["nc"]
    names = ["conv1_w", "conv1_b", "dkc_w", "dkc_b", "dkc_fc_w", "dkc_fc_b",
             "conv2_w", "conv2_b", "conv3_w", "conv3_b"]
    shared = {n: np.ascontiguousarray(np.asarray(inputs[n], dtype=np.float32))
              for n in names}
    x = np.asarray(inputs["x"], dtype=np.float32)
    in_maps = [dict(shared, x=np.ascontiguousarray(x[i])) for i in range(8)]
    r = run_bass_kernel_spmd(nc, in_maps, list(range(8)))
    _NC_CACHE["last_result"] = r
    return np.stack([r.results[i]["out"] for i in range(8)]).astype(np.float32)


# revision 13
# speedup vs baseline: 2.2830x; 1.5190x over previous
"""TRN2 Bass/Tile kernel: deformable-kernel spatial attention (dense_cnn).

Per-core (pure data parallel, batch 8 over 8 cores):
  x cached in SBUF as fp16 with column-parity-split layout (single HBM
  read, contiguous moving-operand reads for the stride-2 conv1 and the
  final attention multiply). Spatial halves packed on partitions 0:64 /
  64:128 so every matmul runs K=128 with block-diagonal weights.
  h1 = relu(conv1(x))     3x3 stride-2 64->64, 9 tap-matmuls, 4-row bands
  5x dkc:                 global-pool -> fc offsets -> hat-function
                          resample of the 4x4 scope kernel -> depthwise
                          3x3: 7 diagonal matmuls on PE + 2 DVE FMAs
  conv2+pixel_shuffle+conv3 fused into a 64->4 channel 3x3 conv
  att replicated across channels with K=8 selector matmuls; final
  out = x * att on DVE from the fp16 x cache.

Queue discipline: sync = x top-half loads + out top-half stores;
gpsimd = x bottom-half loads + out bottom-half stores + replicates;
tensor queue issues its own weight-staging DMAs and halo copies so the
PE stream is gated naturally; V/S do the fp32->fp16 casts for the
top/bottom halves respectively.
"""

import numpy as np

import concourse.bass as bass
import concourse.mybir as mybir
import concourse.tile as tile
from concourse import bacc
from concourse.bass_utils import run_bass_kernel_spmd
from concourse.masks import make_identity
from contextlib import ExitStack

f32 = mybir.dt.float32
f32r = mybir.dt.float32r
f16 = mybir.dt.float16
AF = mybir.ActivationFunctionType
ALU = mybir.AluOpType
AX = mybir.AxisListType

C = 64
H = 256
HH = 128
RB = 64          # interior feature rows per half
SLOTS = RB + 2   # + top/bottom halo row
WCOL = HH + 2    # zero gutter columns at 0 and 129
XSL = 130        # x-cache slots per half: halo, 128 rows, (unused)
XW = 257         # x-cache cols: [zero | odd cols 1..128 | even cols 129..256]
NL = 5
BR = 4           # output rows per band (moving operand max 512 elements)
NB = RB // BR    # bands per conv phase
XCH = 8          # x rows per load chunk (per half)
NCH = 128 // XCH

TAPS = [(t // 3 - 1, t % 3 - 1) for t in range(9)]  # t = 3*ty+tx -> (dy, dx)
PE_TAPS = [0, 1, 2, 3, 4, 6, 8]
DVE_TAPS = [5, 7]
# x-cache column base for a tap reading input col 2*xo+dx
XCB = {-1: 0, 0: 129, 1: 1}


def _ap(a, extra_off, dims):
    return bass.AP(tensor=a.tensor, offset=a.offset + extra_off, ap=dims)


def build_nc():
    nc = bacc.Bacc("TRN2", target_bir_lowering=False, debug=False)
    x_d = nc.dram_tensor("x", [C, H, H], f32, kind="ExternalInput").ap()
    w1_d = nc.dram_tensor("conv1_w", [C, C, 3, 3], f32, kind="ExternalInput").ap()
    b1_d = nc.dram_tensor("conv1_b", [C], f32, kind="ExternalInput").ap()
    dkw_d = nc.dram_tensor("dkc_w", [NL, C, 1, 4, 4], f32, kind="ExternalInput").ap()
    dkb_d = nc.dram_tensor("dkc_b", [NL, C], f32, kind="ExternalInput").ap()
    fcw_d = nc.dram_tensor("dkc_fc_w", [NL, 18, C], f32, kind="ExternalInput").ap()
    fcb_d = nc.dram_tensor("dkc_fc_b", [NL, 18], f32, kind="ExternalInput").ap()
    w2_d = nc.dram_tensor("conv2_w", [4 * C, C, 3, 3], f32, kind="ExternalInput").ap()
    b2_d = nc.dram_tensor("conv2_b", [4 * C], f32, kind="ExternalInput").ap()
    w3_d = nc.dram_tensor("conv3_w", [1, C, 1, 1], f32, kind="ExternalInput").ap()
    b3_d = nc.dram_tensor("conv3_b", [1], f32, kind="ExternalInput").ap()
    o_d = nc.dram_tensor("out", [C, H, H], f32, kind="ExternalOutput").ap()

    with tile.TileContext(nc) as tc:
        with ExitStack() as ctx:
            _kernel(ctx, tc, nc, x_d, w1_d, b1_d, dkw_d, dkb_d, fcw_d, fcb_d,
                    w2_d, b2_d, w3_d, b3_d, o_d)
    nc.compile()
    return nc


def _kernel(ctx, tc, nc, x_d, w1_d, b1_d, dkw_d, dkb_d, fcw_d, fcb_d,
            w2_d, b2_d, w3_d, b3_d, o_d):
    persist = ctx.enter_context(tc.tile_pool(name="persist", bufs=1))
    xcp = ctx.enter_context(tc.tile_pool(name="xcp", bufs=1))
    hpool = ctx.enter_context(tc.tile_pool(name="h", bufs=2))
    stagep = ctx.enter_context(tc.tile_pool(name="stage", bufs=4))
    small = ctx.enter_context(tc.tile_pool(name="small", bufs=4))
    diagp = ctx.enter_context(tc.tile_pool(name="diag", bufs=2))
    outp = ctx.enter_context(tc.tile_pool(name="outb", bufs=2))
    psum = ctx.enter_context(tc.tile_pool(name="psum", bufs=6, space="PSUM"))
    psmall = ctx.enter_context(tc.tile_pool(name="psmall", bufs=2, space="PSUM"))

    # ---------------- early memsets (V) ----------------
    zrow = persist.tile([128, XW], f32)
    nc.vector.memset(zrow[:], 0.0)
    w1t = persist.tile([128, 9, 128], f16)
    nc.vector.memset(w1t[:], 0.0)
    w2ft = persist.tile([128, 9, 8], f16)
    nc.vector.memset(w2ft[:], 0.0)

    # ---------------- weight staging DMAs on the tensor queue ----------------
    w1flat = persist.tile([C, 576], f32)
    nc.scalar.dma_start(out=w1flat[:], in_=_ap(w1_d, 0, [[576, C], [1, 576]]))
    c2wj = persist.tile([C, 4, 576], f32r)
    for j in range(4):
        nc.scalar.dma_start(out=c2wj[:, j, :],
                            in_=_ap(w2_d.bitcast(f32r), j * 576, [[4 * 576, C], [1, 576]]))
    w3sb = persist.tile([C, 4], f32r)
    nc.scalar.dma_start(out=w3sb[:].unsqueeze(-1),
                        in_=_ap(w3_d.bitcast(f32r), 0, [[1, C], [0, 4], [1, 1]]))
    c2bj = persist.tile([C, 4], f32r)
    nc.scalar.dma_start(out=c2bj[:], in_=_ap(b2_d.bitcast(f32r), 0, [[4, C], [1, 4]]))

    # ---------------- gpsimd: tiny iota/identity/selector setup ----------------
    i128 = persist.tile([128, 128], f32)
    make_identity(nc, i128[:])
    it = small.tile([16, 18], mybir.dt.int32, tag="it")
    nc.gpsimd.iota(it[:, 0:9], pattern=[[1, 3], [0, 3]], base=0, channel_multiplier=0)
    nc.gpsimd.iota(it[:, 9:18], pattern=[[0, 3], [1, 3]], base=0, channel_multiplier=0)
    itp = small.tile([16, 1], mybir.dt.int32, tag="itp")
    nc.gpsimd.iota(itp[:], pattern=[[0, 1]], base=0, channel_multiplier=1)
    st16 = small.tile([1, 16], mybir.dt.int32, tag="st16")
    nc.gpsimd.iota(st16[:], pattern=[[1, 4], [0, 4]], base=0, channel_multiplier=0)
    sel8f = persist.tile([8, 4, 128], f32)
    nc.gpsimd.memset(sel8f[:], 0.0)
    for j in range(4):
        for g in range(2):
            nc.gpsimd.affine_select(out=sel8f[:, j, 64 * g:64 * (g + 1)],
                                    in_=sel8f[:, j, 64 * g:64 * (g + 1)],
                                    pattern=[[0, 64]], compare_op=ALU.not_equal,
                                    fill=1.0, base=-(4 * g + j),
                                    channel_multiplier=1)

    # ---------------- conv1 weights via PE transpose -> block-diag fp16 ----------------
    for g3 in range(3):
        tp = psum.tile([128, BR, HH], f32, tag="cps")
        for tt in range(3):
            t = 3 * g3 + tt
            in_t = _ap(w1flat, t, [w1flat.ap[0], [9, C]])
            nc.tensor.transpose(tp[0:C, tt, 0:C], in_t, i128[0:C, 0:C])
        nc.scalar.activation(w1t[0:C, 3 * g3:3 * g3 + 3, 0:C], tp[0:C, 0:3, 0:C],
                             AF.Copy, bias=0.0, scale=1.0)
    nc.scalar.dma_start(out=w1t[C:128, :, C:128], in_=w1t[0:C, :, 0:C])

    # fused conv2/conv3 weights: W2f[ci, t, j] = sum_c conv2_w[4c+j, ci, t] * conv3_w[c]
    w2f_ps = psmall.tile([C, 144], f32, tag="sp")
    for t in range(9):
        for j in range(4):
            lhsT = _ap(c2wj, j * 576 + t, [c2wj.ap[0], [9, C]])
            k = (t * 4 + j) * 4
            nc.tensor.matmul(w2f_ps[:, k:k + 4], lhsT, w3sb[:],
                             start=True, stop=True)
    b2f_ps = psmall.tile([4, 4], f32, tag="sp")
    nc.tensor.matmul(b2f_ps[:], c2bj[:], w3sb[:], start=True, stop=True)

    # remaining weight staging on tensor queue (needed from dkc1 on)
    biases = persist.tile([128, 6], f32)
    nc.scalar.dma_start(out=biases[0:C, 0:1], in_=b1_d.unsqueeze(-1))
    for i in range(NL):
        nc.scalar.dma_start(out=biases[0:C, 1 + i:2 + i], in_=dkb_d[i].unsqueeze(-1))
    w2dt2 = persist.tile([16, NL, 128], f32r)
    for i in range(NL):
        src = _ap(dkw_d.bitcast(f32r), i * 1024, [[1, 16], [16, C]])
        nc.scalar.dma_start(out=w2dt2[:, i, 0:C], in_=src)
        nc.scalar.dma_start(out=w2dt2[:, i, C:128], in_=src)
    fcwtf = persist.tile([128, NL, 18], f32)
    for i in range(NL):
        src = _ap(fcw_d, i * 18 * C, [[1, C], [C, 18]])
        nc.scalar.dma_start(out=fcwtf[0:C, i, :], in_=src)
        nc.scalar.dma_start(out=fcwtf[C:128, i, :], in_=src)
    fcb16 = persist.tile([16, NL, 18], f32)
    nc.scalar.dma_start(out=fcb16[:], in_=_ap(fcb_d, 0, [[0, 16], [18, NL], [1, 18]]))
    b3b8 = small.tile([8, 1], f32, tag="b3b")
    nc.scalar.dma_start(out=b3b8[:], in_=_ap(b3_d, 0, [[0, 8], [1, 1]]))

    # ---------------- x cache gutters (S) ----------------
    xc = xcp.tile([128, XSL, XW], f16)
    zc = _ap(zrow, 0, [zrow.ap[0], [1, XSL], [1, 1]])
    nc.scalar.activation(xc[:, :, 0:1], zc, AF.Copy, bias=0.0, scale=1.0)
    nc.scalar.activation(xc[0:C, 0, :], zrow[0:C, 0:XW], AF.Copy, bias=0.0, scale=1.0)
    nc.scalar.activation(xc[:, XSL - 1, :], zrow[:, 0:XW], AF.Copy, bias=0.0, scale=1.0)

    # ---------------- x load + cast (chunk 15 first for the halo) ----------------
    for ci, k in enumerate([NCH - 1] + list(range(NCH - 1))):
        r0 = XCH * k
        st = stagep.tile([128, XCH, H], f32, tag="xst")
        nc.sync.dma_start(out=st[0:C], in_=x_d[:, r0:r0 + XCH, :])
        nc.gpsimd.dma_start(out=st[C:128], in_=x_d[:, 128 + r0:128 + r0 + XCH, :])
        # V casts the top half, S the bottom half (odd cols -> 1..128, even -> 129..256)
        nc.vector.tensor_copy(xc[0:C, 1 + r0:1 + r0 + XCH, 1:129], st[0:C, :, 1:H:2])
        nc.vector.tensor_copy(xc[0:C, 1 + r0:1 + r0 + XCH, 129:257], st[0:C, :, 0:H:2])
        nc.scalar.activation(xc[C:128, 1 + r0:1 + r0 + XCH, 1:129], st[C:128, :, 1:H:2],
                             AF.Copy, bias=0.0, scale=1.0)
        nc.scalar.activation(xc[C:128, 1 + r0:1 + r0 + XCH, 129:257], st[C:128, :, 0:H:2],
                             AF.Copy, bias=0.0, scale=1.0)
        if k == NCH - 1:
            nc.scalar.dma_start(out=xc[C:128, 0, :], in_=xc[0:C, 128, :])

    # ---------------- h tensors ----------------
    def new_h():
        h = hpool.tile([128, SLOTS, WCOL], f16, tag="h")
        nc.scalar.activation(h[0:C, 0, :], zrow[0:C, 0:WCOL], AF.Copy, bias=0.0, scale=1.0)
        nc.scalar.activation(h[C:128, SLOTS - 1, :], zrow[C:128, 0:WCOL], AF.Copy, bias=0.0, scale=1.0)
        zch = _ap(zrow, 0, [zrow.ap[0], [1, SLOTS], [1, 1]])
        nc.scalar.activation(h[:, :, 0:1], zch, AF.Copy, bias=0.0, scale=1.0)
        nc.scalar.activation(h[:, :, WCOL - 1:WCOL], zch, AF.Copy, bias=0.0, scale=1.0)
        return h

    def halo_fix(h):
        nc.scalar.dma_start(out=h[C:128, 0, :], in_=h[0:C, RB, :])
        nc.scalar.dma_start(out=h[0:C, SLOTS - 1, :], in_=h[C:128, 1, :])

    # ---------------- conv1 (stride 2) ----------------
    h1 = new_h()
    pp1 = small.tile([128, NB], f32, tag="pp")
    for b in range(NB):
        ps = psum.tile([128, BR, HH], f32, tag="cps")
        for t in range(9):
            dy, dx = TAPS[t]
            cb = XCB[dx]
            rhs = xc[:, 1 + 2 * BR * b + dy:1 + 2 * BR * b + dy + 2 * BR - 1:2,
                     cb:cb + HH]
            nc.tensor.matmul(ps[:], w1t[:, t, :], rhs,
                             start=(t == 0), stop=(t == 8))
        s0 = 1 + BR * b
        nc.scalar.activation(h1[:, s0:s0 + BR, 1:HH + 1], ps[:],
                             AF.Relu, bias=biases[:, 0:1], scale=1.0,
                             accum_out=pp1[:, b:b + 1])
    halo_fix(h1)

    # ---------------- chain constants (emitted late; needed from ~dkc1) ----------------
    b18 = persist.tile([16, 18], f32)
    nc.vector.tensor_copy(b18[:], it[:])
    nc.vector.tensor_scalar_add(b18[:], b18[:], 0.5)
    idx16 = small.tile([16, 1], f32, tag="idx16")
    nc.vector.tensor_copy(idx16[:], itp[:])
    stf = small.tile([1, 16], f32, tag="stf")
    nc.vector.tensor_copy(stf[:], st16[:])
    ky16 = persist.tile([16, 1], f32)
    kx16 = persist.tile([16, 1], f32)
    nc.gpsimd.dma_start(out=ky16[:], in_=_ap(stf, 0, [[1, 16], [16, 1]]))
    nc.vector.scalar_tensor_tensor(kx16[:], ky16[:], -4.0, idx16[:], ALU.mult, ALU.add)
    ones1618 = persist.tile([16, 18], f32)
    nc.vector.memset(ones1618[:], 1.0)
    k18 = persist.tile([16, 18], f32)
    nc.vector.tensor_scalar(k18[:, 0:9], ones1618[:, 0:9], ky16[:], None, ALU.mult)
    nc.vector.tensor_scalar(k18[:, 9:18], ones1618[:, 9:18], kx16[:], None, ALU.mult)
    bk18 = persist.tile([16, NL, 18], f32)
    bmk = persist.tile([16, 18], f32)
    nc.vector.tensor_tensor(bmk[:], b18[:], k18[:], ALU.subtract)
    for i in range(NL):
        nc.vector.tensor_tensor(bk18[:, i, :], bmk[:], fcb16[:, i, :], ALU.add)
    kint = persist.tile([16, 10], f32r)
    nc.vector.tensor_copy(kint[:], zrow[0:16, 0:10])
    ones116f = persist.tile([1, 16], f32)
    nc.vector.memset(ones116f, 1.0)
    ones116 = persist.tile([1, 16], f32r)
    nc.vector.tensor_copy(ones116[:], ones116f[:])
    fcwt2 = persist.tile([128, NL, 18], f32r)
    nc.vector.tensor_scalar(fcwt2[:], fcwtf[:], 1.0 / 16384.0, None, ALU.mult)
    # replicates + small staging off the critical queues
    nc.gpsimd.dma_start(out=biases[C:128, :], in_=biases[0:C, :])
    w2fv = _ap(w2f_ps, 0, [w2f_ps.ap[0], [16, 9], [4, 4]])
    nc.scalar.activation(w2ft[0:C, :, 0:4], w2fv, AF.Copy, bias=0.0, scale=1.0)
    nc.gpsimd.dma_start(out=w2ft[C:128, :, 4:8], in_=w2ft[0:C, :, 0:4])
    b2f8 = persist.tile([8, 1], f32)
    nc.scalar.activation(b2f8[0:4], b2f_ps[:, 0:1], AF.Copy, bias=0.0, scale=1.0)
    nc.gpsimd.dma_start(out=b2f8[4:8], in_=b2f8[0:4])
    nc.vector.tensor_add(b2f8[:], b2f8[:], b3b8[:])
    sel8 = persist.tile([8, 4, 128], f16)
    nc.scalar.activation(sel8[:], sel8f[:], AF.Copy, bias=0.0, scale=1.0)
    att8 = persist.tile([8, RB, HH], f16)

    # ---------------- dkc layers ----------------
    h_cur, pp_cur = h1, pp1
    for li in range(NL):
        redr = small.tile([128, 1], f32r, tag="redr")
        with nc.allow_low_precision(reason="f32r is fp32 bits (PE fast-load fmt)"):
            nc.vector.tensor_reduce(out=redr[:], in_=pp_cur[:], axis=AX.X, op=ALU.add)
        offp = psmall.tile([1, 18], f32, tag="sp")
        nc.tensor.matmul(offp[:], redr[:], fcwt2[:, li, :], start=True, stop=True)
        off = small.tile([1, 18], f32r, tag="off")
        nc.vector.tensor_copy(off[:], offp[:])
        offr_ps = psmall.tile([16, 18], f32, tag="sp")
        nc.tensor.matmul(offr_ps[:], ones116[:], off[:], start=True, stop=True)
        # phi = relu(1 - |offp + b - k|) = relu(min(1 - d, 1 + d))
        phi = small.tile([16, 18], f32, tag="phi")
        phia = small.tile([16, 18], f32, tag="phia")
        nc.vector.tensor_tensor(phi[:], offr_ps[:], bk18[:, li, :], ALU.add)
        nc.vector.scalar_tensor_tensor(phia[:], phi[:], 1.0, ones1618[:],
                                       ALU.mult, ALU.add)
        nc.vector.scalar_tensor_tensor(phi[:], phi[:], -1.0, ones1618[:],
                                       ALU.mult, ALU.add)
        nc.vector.tensor_tensor(phi[:], phi[:], phia[:], ALU.min)
        nc.vector.tensor_scalar(phi[:], phi[:], 0.0, None, ALU.max)
        nc.vector.tensor_tensor(kint[:, 0:9], phi[:, 0:9], phi[:, 9:18], ALU.mult)
        samp_ps = psmall.tile([128, 10], f32, tag="sp")
        nc.tensor.matmul(samp_ps[:], w2dt2[:, li, :], kint[:], start=True, stop=True)
        sampf = small.tile([128, 9], f32, tag="samp")
        nc.vector.tensor_copy(sampf[:], samp_ps[:, 0:9])
        diag = diagp.tile([128, 9, 128], f16, tag="diag")
        for i, t in enumerate(PE_TAPS):
            if i % 2 == 0:
                nc.vector.tensor_scalar(diag[:, t, :], i128[:], sampf[:, t:t + 1],
                                        None, ALU.mult)
            else:
                nc.scalar.activation(diag[:, t, :], i128[:], AF.Copy,
                                     bias=0.0, scale=sampf[:, t:t + 1])

        h_nxt = new_h()
        if li < NL - 1:
            pp_nxt = small.tile([128, NB], f32, tag="pp")
        else:
            pp_nxt = None
        for b in range(NB):
            s0 = 1 + BR * b
            ps = psum.tile([128, BR, HH], f32, tag="cps")
            for i, t in enumerate(PE_TAPS):
                dy, dx = TAPS[t]
                rhs = h_cur[:, s0 + dy:s0 + dy + BR, 1 + dx:1 + dx + HH]
                nc.tensor.matmul(ps[:], diag[:, t, :], rhs,
                                 start=(i == 0), stop=(i == len(PE_TAPS) - 1))
            for t in DVE_TAPS:
                dy, dx = TAPS[t]
                rhs = h_cur[:, s0 + dy:s0 + dy + BR, 1 + dx:1 + dx + HH]
                nc.vector.scalar_tensor_tensor(ps[:], rhs, sampf[:, t:t + 1],
                                               ps[:], ALU.mult, ALU.add)
            if pp_nxt is not None:
                nc.scalar.activation(h_nxt[:, s0:s0 + BR, 1:HH + 1], ps[:],
                                     AF.Relu, bias=biases[:, 1 + li:2 + li],
                                     scale=1.0, accum_out=pp_nxt[:, b:b + 1])
            else:
                nc.scalar.activation(h_nxt[:, s0:s0 + BR, 1:HH + 1], ps[:],
                                     AF.Relu, bias=biases[:, 1 + li:2 + li],
                                     scale=1.0)
        halo_fix(h_nxt)
        h_cur, pp_cur = h_nxt, pp_nxt

    # ---------------- fused conv2' -> att; selector replicate; out (pipelined) ----------------
    # conv2' band b covers att rows 4b..4b+3 == output rows 8b..8b+7; the
    # final multiply for band b-1 runs while conv2' band b streams.
    for b in range(NB + 1):
        if b < NB:
            s0 = 1 + BR * b
            ps2 = psum.tile([128, BR, HH], f32, tag="cps")
            for t in range(9):
                dy, dx = TAPS[t]
                rhs = h_cur[:, s0 + dy:s0 + dy + BR, 1 + dx:1 + dx + HH]
                nc.tensor.matmul(ps2[0:8, :, :], w2ft[:, t, :], rhs,
                                 start=(t == 0), stop=(t == 8))
            nc.scalar.activation(att8[:, BR * b:BR * b + BR, :], ps2[0:8, :, :],
                                 AF.Sigmoid, bias=b2f8[:], scale=1.0)
        if b > 0:
            bb = b - 1
            ob = outp.tile([128, 2 * BR, H], f32, tag="ob")
            for j in range(4):
                dy, dx = j // 2, j % 2
                pa = psum.tile([128, BR, HH], f32, tag="cps")
                nc.tensor.matmul(pa[:], sel8[:, j, :],
                                 att8[0:8, BR * bb:BR * bb + BR, :],
                                 start=True, stop=True, skip_group_check=True)
                cb = 129 if dx == 0 else 1
                xv = xc[:, 1 + 2 * BR * bb + dy:1 + 2 * BR * bb + dy + 2 * BR - 1:2,
                        cb:cb + HH]
                ov = _ap(ob, dy * H + dx, [ob.ap[0], [2 * H, BR], [2, HH]])
                nc.vector.tensor_tensor(ov, xv, pa[:], ALU.mult)
            y0 = 2 * BR * bb
            nc.sync.dma_start(out=o_d[:, y0:y0 + 2 * BR, :], in_=ob[0:C])
            nc.gpsimd.dma_start(out=o_d[:, 128 + y0:128 + y0 + 2 * BR, :], in_=ob[C:128])


_NC_CACHE = {}


def kernel(**inputs):
    if "nc" not in _NC_CACHE:
        _NC_CACHE["nc"] = build_nc()
    nc = _NC_CACHE["nc"]
    names = ["conv1_w", "conv1_b", "dkc_w", "dkc_b", "dkc_fc_w", "dkc_fc_b",
             "conv2_w", "conv2_b", "conv3_w", "conv3_b"]
    shared = {n: np.ascontiguousarray(np.asarray(inputs[n], dtype=np.float32))
              for n in names}
    x = np.asarray(inputs["x"], dtype=np.float32)
    in_maps = [dict(shared, x=np.ascontiguousarray(x[i])) for i in range(8)]
    r = run_bass_kernel_spmd(nc, in_maps, list(range(8)))
    _NC_CACHE["last_result"] = r
    return np.stack([r.results[i]["out"] for i in range(8)]).astype(np.float32)
